# revision 22
# baseline (speedup 1.0000x reference)
"""AdaptiveMultiWIRE Trainium2 kernel (8 NeuronCores, SPMD data-parallel over
selected channels).

Math per selected channel c (see reference):
  L0: lin = x@w0+b0, lin2 = x@w0o+b0o (real);  h0 = exp(i*30*lin - 100*(lin^2+lin2^2))
  L1: l1 = h0@W1+b1, l2 = h0@W2+b2 (complex, W = (U@V).reshape(181,181))
      h1 = exp(i*30*l1 - 100*(|l1|^2+|l2|^2))
  out = h1@Wf + bf   (complex, (2048, 3))

Key structure:
  - all matmuls in (hid, pts) orientation: psum = W_lhsT.T @ h_rhs, N=512.
  - lo rows (hid 128:181) of the h planes are PACKED into one [118, pts] tile:
    rows 0:53 = re_lo, rows 64:117 = im_lo, row 117 = ones (bias row), rows
    53:64 = 1.0 (paired with zero stationary rows - harmless).  3 accumulation
    matmuls per psum instead of 4.
  - single-stage range reduction: psum = 30*lin; sin arg = wrap(psum, 0, pi,
    2pi); cos arg = wrap(psum, pi/2, pi, 2pi); elements beyond +-3pi coverage
    have exp(-100 lin^2) ~ 0 so the wrong trig value is annihilated.
  - sin/cos args col-packed -> one Sin op per (ch, layer) for hi and for lo.
  - W-build overlapped with group 0's L0 phase (no barrier); U lhsT built via
    DRAM-transpose DMAs (no PE transpose / extra psum bank).
"""

import numpy as np

NCORES = 8
NCH, NSEL, NPTS, INF, HID, OUT = 128, 64, 2048, 2, 181, 3
C = NSEL // NCORES  # channels per core
HH, HL = 128, HID - 128  # 128 / 53
KP = 118  # packed lo tile rows: 0:53 re_lo, 64:117 im_lo, 117 ones
PT = 512
NT = NPTS // PT
PI = float(np.pi)
OM = 30.0
GROUP_SIZE = 2
VCHUNK = 181 * 32  # 5792

_CACHE = {}

# output o -> (m_a, m_b): psum_o = W[m_a].T @ h_re + W[m_b].T @ h_im (+bias of m_a)
# m blocks: 0=30*W1re, 1=30*W1im, 2=-30*W1im, 3=30*W2re, 4=30*W2im, 5=-30*W2im
O_MAP = [(0, 2), (1, 0), (3, 5), (4, 3)]


def _build():
    import concourse.bass as bass
    from concourse import bacc
    import concourse.mybir as mybir
    import concourse.tile as tile
    from concourse.tile import add_dep_helper

    dt = mybir.dt
    AF = mybir.ActivationFunctionType
    F32, F16, I32 = dt.float32, dt.float16, dt.int32
    F32R = dt.float32r

    nc = bacc.Bacc("TRN2", target_bir_lowering=False, debug=False, num_devices=NCORES)

    # ---------------- DRAM parameters ----------------
    xpk = [nc.declare_dram_parameter(f"xpk{j}", [67, NPTS], F16, isOutput=False)
           for j in range(3)]
    idx = nc.declare_dram_parameter("idx", [C, 1], I32, isOutput=False)
    w0pack = nc.declare_dram_parameter("w0pack", [NCH, 3 * 384], F16, isOutput=False)
    upack = nc.declare_dram_parameter("upack", [NCH, 32], F32, isOutput=False)
    vpack = nc.declare_dram_parameter("vpack", [16, HID * HID + 1], F32R, isOutput=False)
    b1pack = nc.declare_dram_parameter("b1pack", [NCH, 6 * HID], F16, isOutput=False)
    wfpack = nc.declare_dram_parameter("wfpack", [NCH, 2 * 182 * 6], F16, isOutput=False)
    out48 = nc.declare_dram_parameter("out48", [6 * C, NPTS], F32, isOutput=True)

    with tile.TileContext(nc) as tc:
        with (
            tc.tile_pool(name="cpool", bufs=1) as cpool,
            tc.tile_pool(name="wpool", bufs=1) as wpool,
            tc.tile_pool(name="gpool", bufs=1) as gpool,
            tc.tile_pool(name="vpool", bufs=1) as vpool,
            tc.tile_pool(name="stg", bufs=1) as stgpool,
            tc.tile_pool(name="dstage", bufs=1, space="DRAM") as dpool,
            tc.tile_pool(name="planes", bufs=2) as pl2,
            tc.tile_pool(name="planes1", bufs=3) as pl1,
            tc.tile_pool(name="fpool", bufs=1) as fpool,
            tc.tile_pool(name="pmain", bufs=1, space="PSUM") as pmain,
        ):
            # ---------- constants ----------
            def constv(val):
                t = cpool.tile([128, 1], F32, tag=f"const{val}",
                               name=f"const{val}".replace(".", "_").replace("-", "m"))
                nc.vector.memset(t[:], float(val))
                return t

            c_015 = constv(0.15)
            c_225 = constv(2.25)
            bar_a = cpool.tile([1, 1], F32, tag="bar_a")
            bar_b = cpool.tile([1, 1], F32, tag="bar_b")
            nc.vector.memset(bar_a[:], 0.0)
            nc.vector.memset(bar_b[:], 0.0)
            acts_by_phase = [[]]

            def act(op):
                acts_by_phase[-1].append(op)
                return op

            def new_phase():
                acts_by_phase.append([])

            # ---------- prologue: gathers (all early; only need idx) ----------
            idx_t = gpool.tile([C, 1], I32)
            nc.sync.dma_start(idx_t[:], idx[:])

            def gather(table, width, dtp=F16):
                t = gpool.tile([C, width], dtp)
                nc.gpsimd.indirect_dma_start(
                    out=t[:], out_offset=None, in_=table[:],
                    in_offset=bass.IndirectOffsetOnAxis(ap=idx_t[:, :1], axis=0),
                )
                return t

            w0g = gather(w0pack, 3 * 384)
            upg = gather(upack, 32, F32)
            b1g = gather(b1pack, 6 * HID)
            wfg = gather(wfpack, 2 * 182 * 6)

            # ---------- x and w0 stationary tiles ----------
            xsb = [cpool.tile([67, NPTS], F16, tag=f"xsb{j}", name=f"xsb{j}") for j in range(3)]
            for j in range(3):
                nc.sync.dma_start(xsb[j][:], xpk[j][:])
            w0sb = [cpool.tile([67, 384], F16, tag=f"w0sb{j}", name=f"w0sb{j}") for j in range(3)]
            w0g_d = dpool.tile([C, 3 * 384], F16)
            nc.sync.dma_start(w0g_d[:], w0g[:])
            for ch in range(C):
                j, s = ch // 3, ch % 3
                nc.gpsimd.dma_start(
                    w0sb[j][32 * s:32 * s + 3, :],
                    w0g_d[ch:ch + 1, :].rearrange("p (a b) -> (p a) b", a=3),
                )

            # ---------- U lhsT via DRAM transpose DMAs ----------
            # upg cols: [U1re U1im U2re U2im | same negated] (host provides both)
            nc.vector.tensor_scalar_mul(upg[:], upg[:], OM)
            upg_d = dpool.tile([C, 32], F32)
            nc.sync.dma_start(upg_d[:], upg[:])
            ulhsT = cpool.tile([16, 6 * C], F32R, tag="ulhsT")
            zfill = gpool.tile([16, 6 * C], F32)
            nc.vector.memset(zfill[:], 0.0)
            nc.gpsimd.dma_start(ulhsT[:], zfill[:])
            # block spec: (m, dst_rows_start, src_col_start) with cols 0:16 = +U,
            # 16:32 = -U; sub-blocks of 4 (r1re, r1im, r2re, r2im)
            blocks = [
                (0, 0, 0), (0, 4, 16 + 4),
                (1, 0, 4), (1, 4, 0),
                (2, 0, 16 + 4), (2, 4, 16 + 0),
                (3, 8, 8), (3, 12, 16 + 12),
                (4, 8, 12), (4, 12, 8),
                (5, 8, 16 + 12), (5, 12, 16 + 8),
            ]
            for m, dr, sc in blocks:
                nc.gpsimd.dma_start(
                    ulhsT[dr:dr + 4, 8 * m:8 * m + 8],
                    upg_d[:, sc:sc + 4].rearrange("p f -> f p"),
                )

            # ---------- W tile declarations ----------
            Whi = [[wpool.tile([HH, HH], F16, tag=f"Whi{ch}_{m}", name=f"Whi{ch}_{m}")
                    for m in range(6)] for ch in range(C)]
            Wpk = [[wpool.tile([KP, HH], F16, tag=f"Wpk{ch}_{o}", name=f"Wpk{ch}_{o}")
                    for o in range(4)] for ch in range(C)]
            # M-packed lo stationaries: cols 0:53 -> out j128:181 of pair's re,
            # cols 64:117 -> pair's im. pair 0 = l1 (m 0,1,2), pair 1 = l2 (m 3,4,5)
            WlpRe = [[wpool.tile([HH, HH], F16, tag=f"Wre{ch}_{pr}", name=f"Wre{ch}_{pr}")
                      for pr in range(2)] for ch in range(C)]
            WlpIm = [[wpool.tile([HH, HH], F16, tag=f"Wim{ch}_{pr}", name=f"Wim{ch}_{pr}")
                      for pr in range(2)] for ch in range(C)]
            WlpLo = [[wpool.tile([KP, HH], F16, tag=f"Wlo{ch}_{pr}", name=f"Wlo{ch}_{pr}")
                      for pr in range(2)] for ch in range(C)]
            for ch in range(C):
                for o in range(4):
                    nc.vector.memset(Wpk[ch][o][:], 0.0)
                for pr in range(2):
                    nc.vector.memset(WlpRe[ch][pr][:], 0.0)
                    nc.vector.memset(WlpIm[ch][pr][:], 0.0)
                    nc.vector.memset(WlpLo[ch][pr][:], 0.0)
            WfHi = [[wpool.tile([HH, 6], F16, tag=f"Wfh{ch}_{a}", name=f"Wfh{ch}_{a}")
                     for a in range(2)] for ch in range(C)]
            WfPk = [wpool.tile([KP, 6], F16, tag=f"Wfp{ch}", name=f"Wfp{ch}")
                    for ch in range(C)]
            for ch in range(C):
                nc.vector.memset(WfPk[ch][:], 0.0)

            def emit_wbuild():
                """W = U@V build: psums -> fp16 stage -> DRAM -> lhsT tiles."""
                kchunks = [(0, 32), (32, 32), (64, 32), (96, 32), (128, 32), (160, 21)]
                stg_d = dpool.tile([6 * C, 6 * VCHUNK], F16, tag="wstg_d", name="stg_d")
                for ci, (k0, nk) in enumerate(kchunks):
                    F = nk * HID
                    Fm = F + (F & 1)
                    base = k0 * HID
                    vch = vpool.tile([16, VCHUNK // 2], F32R, tag="vch", bufs=2)
                    half_f = VCHUNK // 2
                    stg = stgpool.tile([6 * C, VCHUNK], F16, tag="wstg")
                    subi = 0
                    for hidx in range((Fm + half_f - 1) // half_f):
                        w_ = min(half_f, Fm - hidx * half_f)
                        vh = vpool.tile([16, VCHUNK // 2], F32R, tag="vch", bufs=2)
                        nc.sync.dma_start(vh[:, :w_],
                                          vpack[:, base + hidx * half_f:base + hidx * half_f + w_])
                        for oo in range(0, w_, PT):
                            Fs = min(PT, w_ - oo)
                            Fs2 = Fs - (Fs & 3)
                            o0 = hidx * half_f + oo
                            ps = pmain.tile([6 * C, PT], F32, space="PSUM", tag="wps",
                                            bufs=1, name="wps")
                            nc.tensor.matmul(ps[:, :Fs], ulhsT[:],
                                             vh[:, oo:oo + Fs], start=True, stop=True)
                            if subi % 2 == 0:
                                nc.vector.tensor_copy(stg[:, o0:o0 + Fs2], ps[:, 0:Fs2])
                                if Fs2 < Fs:
                                    nc.scalar.activation(stg[:, o0 + Fs2:o0 + Fs],
                                                         ps[:, Fs2:Fs], AF.Copy)
                            else:
                                nc.scalar.activation(stg[:, o0:o0 + Fs], ps[:, 0:Fs], AF.Copy)
                            subi += 1
                    nc.sync.dma_start(stg_d[:, ci * VCHUNK:ci * VCHUNK + F], stg[:, :F])
                # whole-k rearrange DMAs (stg_d cols are globally (k,j)-contiguous)
                for ch in range(C):
                    srcs = []
                    for m in range(6):
                        row = 8 * m + ch
                        srcs.append(stg_d[row:row + 1, :HID * HID].rearrange(
                            "p (k g) -> (p k) g", k=HID))
                    for m in range(6):
                        nc.sync.dma_start(Whi[ch][m][:, :], srcs[m][0:HH, 0:HH])
                    for o, (ma, mb) in enumerate(O_MAP):
                        nc.sync.dma_start(Wpk[ch][o][0:HL, :], srcs[ma][HH:HID, 0:HH])
                        nc.sync.dma_start(Wpk[ch][o][64:64 + HL, :], srcs[mb][HH:HID, 0:HH])
                    for pr in range(2):
                        ma, mb, mn = (0, 1, 2) if pr == 0 else (3, 4, 5)
                        nc.sync.dma_start(WlpRe[ch][pr][:, 0:HL], srcs[ma][0:HH, HH:HID])
                        nc.sync.dma_start(WlpRe[ch][pr][:, 64:64 + HL], srcs[mb][0:HH, HH:HID])
                        nc.sync.dma_start(WlpIm[ch][pr][:, 0:HL], srcs[mn][0:HH, HH:HID])
                        nc.sync.dma_start(WlpIm[ch][pr][:, 64:64 + HL], srcs[ma][0:HH, HH:HID])
                        nc.sync.dma_start(WlpLo[ch][pr][0:HL, 0:HL], srcs[ma][HH:HID, HH:HID])
                        nc.sync.dma_start(WlpLo[ch][pr][0:HL, 64:64 + HL], srcs[mb][HH:HID, HH:HID])
                        nc.sync.dma_start(WlpLo[ch][pr][64:64 + HL, 0:HL], srcs[mn][HH:HID, HH:HID])
                        nc.sync.dma_start(WlpLo[ch][pr][64:64 + HL, 64:64 + HL], srcs[ma][HH:HID, HH:HID])

                # bias rows into Wpk row 117 (j 0:128) and WlpLo row 117 (j 128:181)
                for ch in range(C):
                    for o, (ma, mb) in enumerate(O_MAP):
                        nc.gpsimd.dma_start(Wpk[ch][o][117:118, :],
                                            b1g[ch:ch + 1, ma * HID:ma * HID + HH])
                    for pr in range(2):
                        ma, mb = (0, 1) if pr == 0 else (3, 4)
                        nc.gpsimd.dma_start(WlpLo[ch][pr][117:118, 0:HL],
                                            b1g[ch:ch + 1, ma * HID + HH:(ma + 1) * HID])
                        nc.gpsimd.dma_start(WlpLo[ch][pr][117:118, 64:64 + HL],
                                            b1g[ch:ch + 1, mb * HID + HH:(mb + 1) * HID])

                # Wf tiles
                wfg_d = dpool.tile([C, 2 * 182 * 6], F16)
                nc.sync.dma_start(wfg_d[:], wfg[:])
                for ch in range(C):
                    for a in range(2):
                        o = a * 182 * 6
                        nc.gpsimd.dma_start(
                            WfHi[ch][a][:],
                            wfg_d[ch:ch + 1, o:o + HH * 6].rearrange("p (k g) -> (p k) g", k=HH))
                    nc.gpsimd.dma_start(
                        WfPk[ch][0:HL, :],
                        wfg_d[ch:ch + 1, HH * 6:HID * 6].rearrange("p (k g) -> (p k) g", k=HL))
                    nc.gpsimd.dma_start(
                        WfPk[ch][64:64 + HL, :],
                        wfg_d[ch:ch + 1, 182 * 6 + HH * 6:182 * 6 + HID * 6].rearrange(
                            "p (k g) -> (p k) g", k=HL))
                    nc.gpsimd.dma_start(
                        WfPk[ch][117:118, :],
                        wfg_d[ch:ch + 1, HID * 6:182 * 6])

            # ---------- main pipeline ----------
            ngroups = C // GROUP_SIZE

            def plane(pool, tag, rows, dtp=F16, bufs=None, cols=NPTS):
                return pool.tile([rows, cols], dtp, tag=tag, name=tag, bufs=bufs)

            hplanes = {}

            def emit_l0_exp(chans, grp):
                for ch in chans:
                    j, s_ = ch // 3, ch % 3
                    b = 32 * s_
                    qacc_h = plane(pl2, "qacc_h", HH, bufs=1)
                    qacc_l = plane(pl2, "qacc_l", HL, bufs=1)
                    qtmp_l = plane(pl2, "qtmp_l", HL, bufs=1)
                    sqp = plane(pl2, "sqp", HH, bufs=1)
                    arg_h = plane(pl2, "arg_h", HH, cols=2 * NPTS)
                    arg_l = plane(pl2, "arg_l", HL, cols=2 * NPTS)
                    e_h = plane(pl2, "e_h", HH)
                    e_l = plane(pl2, "e_l", HL)
                    lh = w0sb[j][b:b + 3, :]
                    for np2 in range(NT // 2):
                        pLo = pmain.tile([HH, 2 * PT], F32, space="PSUM", tag="pb", bufs=1, name="pb")
                        qsl = slice(np2 * 2 * PT, (np2 + 1) * 2 * PT)
                        for half in range(2):
                            nt = np2 * 2 + half
                            sl = slice(nt * PT, (nt + 1) * PT)
                            sl2 = slice(NPTS + nt * PT, NPTS + (nt + 1) * PT)
                            hsl = slice(half * PT, (half + 1) * PT)
                            pH = pmain.tile([HH, 2 * PT], F32, space="PSUM", tag="pa", bufs=1, name="pa")
                            rhs = xsb[j][b:b + 3, sl]
                            nc.tensor.matmul(pH[:, 0:PT], lh[:, 0:HH], rhs, start=True, stop=True)
                            nc.tensor.matmul(pH[:, PT:2 * PT], lh[:, HH:2 * HH], rhs, start=True, stop=True)
                            nc.tensor.matmul(pLo[0:117, hsl], lh[:, 256:373], rhs, start=True, stop=True)
                            so = half * 2 * PT
                            act(nc.scalar.activation(sqp[:, so:so + 2 * PT], pH[:], AF.Square,
                                                     scale=1.0 / OM))
                            nc.vector.add_range_wrap(arg_h[:, sl], pH[:, 0:PT], 0.0, PI, 2 * PI)
                            nc.vector.add_range_wrap(arg_h[:, sl2], pH[:, 0:PT], PI / 2, PI, 2 * PI)
                            nc.vector.add_range_wrap(arg_l[:, sl], pLo[0:HL, hsl], 0.0, PI, 2 * PI)
                            nc.vector.add_range_wrap(arg_l[:, sl2], pLo[0:HL, hsl], PI / 2, PI, 2 * PI)
                            nc.vector.tensor_add(qacc_h[:, sl], sqp[:, so:so + PT],
                                                 sqp[:, so + PT:so + 2 * PT])
                        act(nc.scalar.activation(qacc_l[:, qsl], pLo[0:HL, :], AF.Square,
                                                 scale=1.0 / OM))
                        act(nc.scalar.activation(qtmp_l[:, qsl], pLo[64:64 + HL, :], AF.Square,
                                                 scale=1.0 / OM))
                        nc.vector.tensor_add(qacc_l[:, qsl], qacc_l[:, qsl], qtmp_l[:, qsl])
                    act(nc.scalar.activation(e_h[:], qacc_h[:], AF.Exp, scale=-100.0))
                    act(nc.scalar.activation(e_l[:], qacc_l[:], AF.Exp, scale=-100.0))
                    grp[ch] = (arg_h, arg_l, e_h, e_l)

            def emit_l0_trig(chans, grp):
                for ch in chans:
                    arg_h, arg_l, e_h, e_l = grp[ch]
                    sc_h = plane(pl2, "sc_h", HH, bufs=1, cols=2 * NPTS)
                    sc_l = plane(pl2, "sc_l", HL, bufs=1, cols=2 * NPTS)
                    h0re_h = plane(pl1, "h0re_h", HH)
                    h0im_h = plane(pl1, "h0im_h", HH)
                    h0lo = plane(pl1, "h0lo", KP)
                    nc.vector.memset(h0lo[:], 1.0)
                    act(nc.scalar.activation(sc_h[:], arg_h[:], AF.Sin))
                    act(nc.scalar.activation(sc_l[:], arg_l[:], AF.Sin))
                    nc.vector.tensor_mul(h0re_h[:], e_h[:], sc_h[:, NPTS:])
                    nc.vector.tensor_mul(h0im_h[:], e_h[:], sc_h[:, 0:NPTS])
                    nc.vector.tensor_mul(h0lo[0:HL, :], e_l[:], sc_l[:, NPTS:])
                    nc.vector.tensor_mul(h0lo[64:64 + HL, :], e_l[:], sc_l[:, 0:NPTS])
                    hplanes[ch] = (h0re_h, h0im_h, h0lo)

            def emit_l1_exp(chans, grp1):
                for ch in chans:
                    h0re_h, h0im_h, h0lo = hplanes[ch]
                    qacc_h = plane(pl2, "qacc_h", HH, bufs=1)
                    qacc_l = plane(pl2, "qacc_l", HL, bufs=1)
                    qtmp_l = plane(pl2, "qtmp_l", HL, bufs=1)
                    sqp = plane(pl2, "sqp", HH, bufs=1)
                    sql_a = plane(pl2, "sql_a", HL, bufs=1, cols=NPTS // 2)
                    sql_b = plane(pl2, "sql_b", HL, bufs=1, cols=NPTS // 2)
                    arg_h = plane(pl2, "arg_h", HH, cols=2 * NPTS)
                    arg_l = plane(pl2, "arg_l", HL, cols=2 * NPTS)
                    e_h = plane(pl2, "e_h", HH)
                    e_l = plane(pl2, "e_l", HL)

                    def mm3(psum, o, rhs_sl):
                        m_a, m_b = O_MAP[o]
                        nc.tensor.matmul(psum, Whi[ch][m_a][:], h0re_h[:, rhs_sl],
                                         start=True, stop=False)
                        nc.tensor.matmul(psum, Whi[ch][m_b][:], h0im_h[:, rhs_sl],
                                         start=False, stop=False)
                        nc.tensor.matmul(psum, Wpk[ch][o][:], h0lo[:, rhs_sl],
                                         start=False, stop=True)

                    def mm3lo(psum, pr, rhs_sl):
                        nc.tensor.matmul(psum, WlpRe[ch][pr][:], h0re_h[:, rhs_sl],
                                         start=True, stop=False)
                        nc.tensor.matmul(psum, WlpIm[ch][pr][:], h0im_h[:, rhs_sl],
                                         start=False, stop=False)
                        nc.tensor.matmul(psum, WlpLo[ch][pr][:], h0lo[:, rhs_sl],
                                         start=False, stop=True)

                    for nt in range(NT):
                        sl = slice(nt * PT, (nt + 1) * PT)
                        sl2 = slice(NPTS + nt * PT, NPTS + (nt + 1) * PT)
                        pH = pmain.tile([HH, 2 * PT], F32, space="PSUM", tag="pa", bufs=1, name="pa")
                        pH2 = pmain.tile([HH, 2 * PT], F32, space="PSUM", tag="pc", bufs=1, name="pc")
                        pLo = pmain.tile([HH, 2 * PT], F32, space="PSUM", tag="pb", bufs=1, name="pb")
                        mm3(pH[:, 0:PT], 0, sl)
                        mm3(pH[:, PT:2 * PT], 1, sl)
                        mm3lo(pLo[0:HH, 0:PT], 0, sl)
                        mm3lo(pLo[0:HH, PT:2 * PT], 1, sl)
                        mm3(pH2[:, 0:PT], 2, sl)
                        mm3(pH2[:, PT:2 * PT], 3, sl)
                        act(nc.scalar.activation(sqp[:, 0:2 * PT], pH[:], AF.Square,
                                                 scale=1.0 / OM))
                        nc.vector.add_range_wrap(arg_h[:, sl], pH[:, 0:PT], 0.0, PI, 2 * PI)
                        nc.vector.add_range_wrap(arg_h[:, sl2], pH[:, 0:PT], PI / 2, PI, 2 * PI)
                        nc.vector.add_range_wrap(arg_l[:, sl], pLo[0:HL, 0:PT], 0.0, PI, 2 * PI)
                        nc.vector.add_range_wrap(arg_l[:, sl2], pLo[0:HL, 0:PT], PI / 2, PI, 2 * PI)
                        act(nc.scalar.activation(sql_a[:, :], pLo[0:HL, :], AF.Square,
                                                 scale=1.0 / OM))
                        act(nc.scalar.activation(sql_b[:, :], pLo[64:64 + HL, :], AF.Square,
                                                 scale=1.0 / OM))
                        act(nc.scalar.activation(sqp[:, 2 * PT:4 * PT], pH2[:], AF.Square,
                                                 scale=1.0 / OM))
                        nc.vector.tensor_add(qacc_h[:, sl], sqp[:, 0:PT], sqp[:, PT:2 * PT])
                        nc.vector.tensor_add(qacc_h[:, sl], qacc_h[:, sl], sqp[:, 2 * PT:3 * PT])
                        nc.vector.tensor_add(qacc_h[:, sl], qacc_h[:, sl], sqp[:, 3 * PT:4 * PT])
                        nc.vector.tensor_add(qacc_l[:, sl], sql_a[:, 0:PT], sql_a[:, PT:2 * PT])
                        nc.vector.tensor_add(qtmp_l[:, sl], sql_b[:, 0:PT], sql_b[:, PT:2 * PT])
                        nc.vector.tensor_add(qacc_l[:, sl], qacc_l[:, sl], qtmp_l[:, sl])
                    act(nc.scalar.activation(e_h[:], qacc_h[:], AF.Exp,
                                         scale=-100.0, bias=c_225[:HH, :1]))
                    act(nc.scalar.activation(e_l[:], qacc_l[:], AF.Exp,
                                         scale=-100.0, bias=c_225[:HL, :1]))
                    grp1[ch] = (arg_h, arg_l, e_h, e_l)

            def emit_l1_trig(chans, grp1):
                for ch in chans:
                    arg_h, arg_l, e_h, e_l = grp1[ch]
                    sc_h = plane(pl2, "sc_h", HH, bufs=1, cols=2 * NPTS)
                    sc_l = plane(pl2, "sc_l", HL, bufs=1, cols=2 * NPTS)
                    h1re_h = plane(pl1, "h0re_h", HH)
                    h1im_h = plane(pl1, "h0im_h", HH)
                    h1lo = plane(pl1, "h0lo", KP)
                    nc.vector.memset(h1lo[:], 1.0)
                    act(nc.scalar.activation(sc_h[:], arg_h[:], AF.Sin))
                    act(nc.scalar.activation(sc_l[:], arg_l[:], AF.Sin))
                    nc.vector.tensor_mul(h1re_h[:], e_h[:], sc_h[:, NPTS:])
                    nc.vector.tensor_mul(h1im_h[:], e_h[:], sc_h[:, 0:NPTS])
                    nc.vector.tensor_mul(h1lo[0:HL, :], e_l[:], sc_l[:, NPTS:])
                    nc.vector.tensor_mul(h1lo[64:64 + HL, :], e_l[:], sc_l[:, 0:NPTS])
                    for nt in range(NT):
                        sl = slice(nt * PT, (nt + 1) * PT)
                        pf = pmain.tile([6, PT], F32, space="PSUM", tag="fin", bufs=1, name="pf")
                        nc.tensor.matmul(pf[:], WfHi[ch][0][:], h1re_h[:, sl], start=True, stop=False)
                        nc.tensor.matmul(pf[:], WfHi[ch][1][:], h1im_h[:, sl], start=False, stop=False)
                        nc.tensor.matmul(pf[:], WfPk[ch][:], h1lo[:, sl], start=False, stop=True)
                        fs = fpool.tile([6, PT], F32, tag="fstage")
                        nc.vector.tensor_copy(fs[:], pf[:])
                        nc.sync.dma_start(out48[6 * ch:6 * ch + 6, sl], fs[:])

            # emission order: g0 L0 -> W-build -> g0 trig/L1/... -> g1 ...
            groups = [list(range(g * GROUP_SIZE, (g + 1) * GROUP_SIZE))
                      for g in range(ngroups)]
            grps = [dict() for _ in range(ngroups)]
            grps1 = [dict() for _ in range(ngroups)]

            emit_l0_exp(groups[0], grps[0])
            emit_wbuild()
            new_phase()
            emit_l0_trig(groups[0], grps[0])
            for g in range(ngroups):
                if g > 0:
                    new_phase()
                    emit_l0_exp(groups[g], grps[g])
                    new_phase()
                    emit_l0_trig(groups[g], grps[g])
                new_phase()
                emit_l1_exp(groups[g], grps1[g])
                new_phase()
                emit_l1_trig(groups[g], grps1[g])

            # ---------- ACT phase barriers (force table-set grouping) ----------
            bars = []
            for p in range(0):
                if p % 2 == 0:
                    b = nc.scalar.copy(bar_b[:], bar_a[:])
                else:
                    b = nc.scalar.copy(bar_a[:], bar_b[:])
                bars.append(b)
            for p, b in enumerate(bars):
                for op in acts_by_phase[p]:
                    add_dep_helper(b.ins, op.ins, sync=False, reason=f"phase{p}end")
                for op in acts_by_phase[p + 1]:
                    add_dep_helper(op.ins, b.ins, sync=False, reason=f"phase{p+1}start")

    nc.compile()
    return nc


def _prep(inputs):
    x = np.ascontiguousarray(inputs["x"], dtype=np.float32)
    indices = np.ascontiguousarray(inputs["indices"], dtype=np.int32)
    w0_lin = np.asarray(inputs["w0_lin"], dtype=np.float32)
    b0_lin = np.asarray(inputs["b0_lin"], dtype=np.float32)
    w0_orth = np.asarray(inputs["w0_orth"], dtype=np.float32)
    b0_orth = np.asarray(inputs["b0_orth"], dtype=np.float32)
    U1_lin = np.asarray(inputs["U1_lin"], dtype=np.complex64)
    V1_lin = np.asarray(inputs["V1_lin"], dtype=np.complex64)
    b1_lin = np.asarray(inputs["b1_lin"], dtype=np.complex64)
    U1_orth = np.asarray(inputs["U1_orth"], dtype=np.complex64)
    V1_orth = np.asarray(inputs["V1_orth"], dtype=np.complex64)
    b1_orth = np.asarray(inputs["b1_orth"], dtype=np.complex64)
    Wf = np.asarray(inputs["Wf"], dtype=np.complex64)
    bf = np.asarray(inputs["bf"], dtype=np.complex64)

    w0pack = np.zeros((NCH, 3, 384), np.float32)
    w0pack[:, 0:2, 0:HH] = w0_lin[:, :, 0:HH]
    w0pack[:, 2, 0:HH] = b0_lin[:, 0, 0:HH]
    w0pack[:, 0:2, 128:128 + HH] = w0_orth[:, :, 0:HH]
    w0pack[:, 2, 128:128 + HH] = b0_orth[:, 0, 0:HH]
    w0pack[:, 0:2, 256:256 + HL] = w0_lin[:, :, HH:HID]
    w0pack[:, 2, 256:256 + HL] = b0_lin[:, 0, HH:HID]
    w0pack[:, 0:2, 320:320 + HL] = w0_orth[:, :, HH:HID]
    w0pack[:, 2, 320:320 + HL] = b0_orth[:, 0, HH:HID]
    w0pack = (30.0 * w0pack).reshape(NCH, 3 * 384).astype(np.float16)

    up = np.concatenate([U1_lin.real, U1_lin.imag, U1_orth.real, U1_orth.imag],
                        axis=1).astype(np.float32)  # (128, 16)
    upack = np.concatenate([up, -up], axis=1)  # (128, 32)
    vpack = np.concatenate([V1_lin.real, V1_lin.imag, V1_orth.real, V1_orth.imag],
                           axis=0).astype(np.float32)  # (16, 32761)
    vpack = np.concatenate([vpack, np.zeros((16, 1), np.float32)], axis=1)
    z = np.zeros_like(b1_lin[:, 0, :].real)
    b1pack = 30.0 * np.concatenate(
        [b1_lin[:, 0, :].real, b1_lin[:, 0, :].imag + 0.15, z,
         b1_orth[:, 0, :].real, b1_orth[:, 0, :].imag, z],
        axis=1).astype(np.float16)  # (128, 1086)

    wfpack = np.zeros((NCH, 2, 182, 6), np.float32)
    wfpack[:, 0, 0:HID, 0:3] = Wf.real
    wfpack[:, 0, 0:HID, 3:6] = Wf.imag
    wfpack[:, 0, HID, 0:3] = bf[:, 0, :].real
    wfpack[:, 0, HID, 3:6] = bf[:, 0, :].imag
    wfpack[:, 1, 0:HID, 0:3] = -Wf.imag
    wfpack[:, 1, 0:HID, 3:6] = Wf.real
    wfpack = wfpack.reshape(NCH, 2 * 182 * 6).astype(np.float16)

    in_maps = []
    for core in range(NCORES):
        c0 = core * C
        xs = x[c0:c0 + C]
        xpk = [np.zeros((67, NPTS), np.float16) for _ in range(3)]
        for ch in range(C):
            j, s = ch // 3, ch % 3
            xpk[j][32 * s:32 * s + 2, :] = xs[ch].T
            xpk[j][32 * s + 2, :] = 1.0
        m = {f"xpk{j}": xpk[j] for j in range(3)}
        m["idx"] = indices[c0:c0 + C].reshape(C, 1)
        m["w0pack"] = w0pack
        m["upack"] = upack
        m["vpack"] = vpack
        m["b1pack"] = b1pack
        m["wfpack"] = wfpack
        in_maps.append(m)
    return in_maps


def kernel(**inputs):
    from concourse import bass_utils
    if "nc" not in _CACHE:
        _CACHE["nc"] = _build()
    nc = _CACHE["nc"]
    in_maps = _prep(inputs)
    res = bass_utils.run_bass_kernel_spmd(nc, in_maps, core_ids=list(range(NCORES)))
    out = np.zeros((NSEL, NPTS, OUT), np.complex64)
    for core in range(NCORES):
        o = res.results[core]["out48"]
        for ch in range(C):
            re = o[6 * ch:6 * ch + 3, :]
            im = o[6 * ch + 3:6 * ch + 6, :]
            out[core * C + ch] = (re + 1j * im).T.astype(np.complex64)
    return out


# revision 23
# speedup vs baseline: 1.0357x; 1.0357x over previous
"""AdaptiveMultiWIRE Trainium2 kernel (8 NeuronCores, SPMD data-parallel over
selected channels).

Math per selected channel c (see reference):
  L0: lin = x@w0+b0, lin2 = x@w0o+b0o (real);  h0 = exp(i*30*lin - 100*(lin^2+lin2^2))
  L1: l1 = h0@W1+b1, l2 = h0@W2+b2 (complex, W = (U@V).reshape(181,181))
      h1 = exp(i*30*l1 - 100*(|l1|^2+|l2|^2))
  out = h1@Wf + bf   (complex, (2048, 3))

Key structure:
  - all matmuls in (hid, pts) orientation: psum = W_lhsT.T @ h_rhs, N=512.
  - lo rows (hid 128:181) of the h planes are PACKED into one [118, pts] tile:
    rows 0:53 = re_lo, rows 64:117 = im_lo, row 117 = ones (bias row), rows
    53:64 = 1.0 (paired with zero stationary rows - harmless).  3 accumulation
    matmuls per psum instead of 4.
  - single-stage range reduction: psum = 30*lin; sin arg = wrap(psum, 0, pi,
    2pi); cos arg = wrap(psum, pi/2, pi, 2pi); elements beyond +-3pi coverage
    have exp(-100 lin^2) ~ 0 so the wrong trig value is annihilated.
  - sin/cos args col-packed -> one Sin op per (ch, layer) for hi and for lo.
  - W-build overlapped with group 0's L0 phase (no barrier); U lhsT built via
    DRAM-transpose DMAs (no PE transpose / extra psum bank).
"""

import numpy as np

NCORES = 8
NCH, NSEL, NPTS, INF, HID, OUT = 128, 64, 2048, 2, 181, 3
C = NSEL // NCORES  # channels per core
HH, HL = 128, HID - 128  # 128 / 53
KP = 118  # packed lo tile rows: 0:53 re_lo, 64:117 im_lo, 117 ones
PT = 512
NT = NPTS // PT
PI = float(np.pi)
OM = 30.0
GROUP_SIZE = 4
VCHUNK = 181 * 32  # 5792

_CACHE = {}

# output o -> (m_a, m_b): psum_o = W[m_a].T @ h_re + W[m_b].T @ h_im (+bias of m_a)
# m blocks: 0=30*W1re, 1=30*W1im, 2=-30*W1im, 3=30*W2re, 4=30*W2im, 5=-30*W2im
O_MAP = [(0, 2), (1, 0), (3, 5), (4, 3)]


def _build():
    import concourse.bass as bass
    from concourse import bacc
    import concourse.mybir as mybir
    import concourse.tile as tile
    from concourse.tile import add_dep_helper

    dt = mybir.dt
    AF = mybir.ActivationFunctionType
    F32, F16, I32 = dt.float32, dt.float16, dt.int32
    F32R = dt.float32r

    nc = bacc.Bacc("TRN2", target_bir_lowering=False, debug=False, num_devices=NCORES)

    # ---------------- DRAM parameters ----------------
    xpk = [nc.declare_dram_parameter(f"xpk{j}", [67, NPTS], F16, isOutput=False)
           for j in range(3)]
    idx = nc.declare_dram_parameter("idx", [C, 1], I32, isOutput=False)
    w0pack = nc.declare_dram_parameter("w0pack", [NCH, 3 * 384], F16, isOutput=False)
    upack = nc.declare_dram_parameter("upack", [NCH, 32], F32, isOutput=False)
    vpack = nc.declare_dram_parameter("vpack", [16, HID * HID + 1], F32R, isOutput=False)
    b1pack = nc.declare_dram_parameter("b1pack", [NCH, 6 * HID], F16, isOutput=False)
    wfpack = nc.declare_dram_parameter("wfpack", [NCH, 2 * 182 * 6], F16, isOutput=False)
    out48 = nc.declare_dram_parameter("out48", [6 * C, NPTS], F32, isOutput=True)

    with tile.TileContext(nc) as tc:
        with (
            tc.tile_pool(name="cpool", bufs=1) as cpool,
            tc.tile_pool(name="wpool", bufs=1) as wpool,
            tc.tile_pool(name="gpool", bufs=1) as gpool,
            tc.tile_pool(name="vpool", bufs=1) as vpool,
            tc.tile_pool(name="stg", bufs=1) as stgpool,
            tc.tile_pool(name="dstage", bufs=1, space="DRAM") as dpool,
            tc.tile_pool(name="planes", bufs=2) as pl2,
            tc.tile_pool(name="planes1", bufs=3) as pl1,
            tc.tile_pool(name="fpool", bufs=1) as fpool,
            tc.tile_pool(name="pmain", bufs=1, space="PSUM") as pmain,
        ):
            # ---------- constants ----------
            def constv(val):
                t = cpool.tile([128, 1], F32, tag=f"const{val}",
                               name=f"const{val}".replace(".", "_").replace("-", "m"))
                nc.vector.memset(t[:], float(val))
                return t

            c_015 = constv(0.15)
            c_225 = constv(2.25)
            bar_a = cpool.tile([1, 1], F32, tag="bar_a")
            bar_b = cpool.tile([1, 1], F32, tag="bar_b")
            nc.vector.memset(bar_a[:], 0.0)
            nc.vector.memset(bar_b[:], 0.0)
            acts_by_phase = [[]]

            def act(op):
                acts_by_phase[-1].append(op)
                return op

            def new_phase():
                acts_by_phase.append([])

            # ---------- prologue: gathers (all early; only need idx) ----------
            idx_t = gpool.tile([C, 1], I32)
            nc.sync.dma_start(idx_t[:], idx[:])

            def gather(table, width, dtp=F16):
                t = gpool.tile([C, width], dtp)
                nc.gpsimd.indirect_dma_start(
                    out=t[:], out_offset=None, in_=table[:],
                    in_offset=bass.IndirectOffsetOnAxis(ap=idx_t[:, :1], axis=0),
                )
                return t

            w0g = gather(w0pack, 3 * 384)
            upg = gather(upack, 32, F32)
            b1g = gather(b1pack, 6 * HID)
            wfg = gather(wfpack, 2 * 182 * 6)

            # ---------- x and w0 stationary tiles ----------
            xsb = [cpool.tile([67, NPTS], F16, tag=f"xsb{j}", name=f"xsb{j}") for j in range(3)]
            for j in range(3):
                nc.sync.dma_start(xsb[j][:], xpk[j][:])
            w0sb = [cpool.tile([67, 384], F16, tag=f"w0sb{j}", name=f"w0sb{j}") for j in range(3)]
            w0g_d = dpool.tile([C, 3 * 384], F16)
            nc.sync.dma_start(w0g_d[:], w0g[:])
            for ch in range(C):
                j, s = ch // 3, ch % 3
                nc.gpsimd.dma_start(
                    w0sb[j][32 * s:32 * s + 3, :],
                    w0g_d[ch:ch + 1, :].rearrange("p (a b) -> (p a) b", a=3),
                )

            # ---------- U lhsT via DRAM transpose DMAs ----------
            # upg cols: [U1re U1im U2re U2im | same negated] (host provides both)
            nc.vector.tensor_scalar_mul(upg[:], upg[:], OM)
            upg_d = dpool.tile([C, 32], F32)
            nc.sync.dma_start(upg_d[:], upg[:])
            ulhsT = cpool.tile([16, 6 * C], F32R, tag="ulhsT")
            zfill = gpool.tile([16, 6 * C], F32)
            nc.vector.memset(zfill[:], 0.0)
            nc.gpsimd.dma_start(ulhsT[:], zfill[:])
            # block spec: (m, dst_rows_start, src_col_start) with cols 0:16 = +U,
            # 16:32 = -U; sub-blocks of 4 (r1re, r1im, r2re, r2im)
            blocks = [
                (0, 0, 0), (0, 4, 16 + 4),
                (1, 0, 4), (1, 4, 0),
                (2, 0, 16 + 4), (2, 4, 16 + 0),
                (3, 8, 8), (3, 12, 16 + 12),
                (4, 8, 12), (4, 12, 8),
                (5, 8, 16 + 12), (5, 12, 16 + 8),
            ]
            for m, dr, sc in blocks:
                nc.gpsimd.dma_start(
                    ulhsT[dr:dr + 4, 8 * m:8 * m + 8],
                    upg_d[:, sc:sc + 4].rearrange("p f -> f p"),
                )

            # ---------- W tile declarations ----------
            Whi = [[wpool.tile([HH, HH], F16, tag=f"Whi{ch}_{m}", name=f"Whi{ch}_{m}")
                    for m in range(6)] for ch in range(C)]
            Wpk = [[wpool.tile([KP, HH], F16, tag=f"Wpk{ch}_{o}", name=f"Wpk{ch}_{o}")
                    for o in range(4)] for ch in range(C)]
            # M-packed lo stationaries: cols 0:53 -> out j128:181 of pair's re,
            # cols 64:117 -> pair's im. pair 0 = l1 (m 0,1,2), pair 1 = l2 (m 3,4,5)
            WlpRe = [[wpool.tile([HH, HH], F16, tag=f"Wre{ch}_{pr}", name=f"Wre{ch}_{pr}")
                      for pr in range(2)] for ch in range(C)]
            WlpIm = [[wpool.tile([HH, HH], F16, tag=f"Wim{ch}_{pr}", name=f"Wim{ch}_{pr}")
                      for pr in range(2)] for ch in range(C)]
            WlpLo = [[wpool.tile([KP, HH], F16, tag=f"Wlo{ch}_{pr}", name=f"Wlo{ch}_{pr}")
                      for pr in range(2)] for ch in range(C)]
            for ch in range(C):
                for o in range(4):
                    nc.vector.memset(Wpk[ch][o][:], 0.0)
                for pr in range(2):
                    nc.vector.memset(WlpRe[ch][pr][:], 0.0)
                    nc.vector.memset(WlpIm[ch][pr][:], 0.0)
                    nc.vector.memset(WlpLo[ch][pr][:], 0.0)
            WfHi = [[wpool.tile([HH, 6], F16, tag=f"Wfh{ch}_{a}", name=f"Wfh{ch}_{a}")
                     for a in range(2)] for ch in range(C)]
            WfPk = [wpool.tile([KP, 6], F16, tag=f"Wfp{ch}", name=f"Wfp{ch}")
                    for ch in range(C)]
            for ch in range(C):
                nc.vector.memset(WfPk[ch][:], 0.0)

            def emit_wbuild():
                """W = U@V build: psums -> fp16 stage -> DRAM -> lhsT tiles."""
                kchunks = [(0, 32), (32, 32), (64, 32), (96, 32), (128, 32), (160, 21)]
                stg_d = dpool.tile([6 * C, 6 * VCHUNK], F16, tag="wstg_d", name="stg_d")
                for ci, (k0, nk) in enumerate(kchunks):
                    F = nk * HID
                    Fm = F + (F & 1)
                    base = k0 * HID
                    vch = vpool.tile([16, VCHUNK // 2], F32R, tag="vch", bufs=2)
                    half_f = VCHUNK // 2
                    stg = stgpool.tile([6 * C, VCHUNK], F16, tag="wstg")
                    subi = 0
                    for hidx in range((Fm + half_f - 1) // half_f):
                        w_ = min(half_f, Fm - hidx * half_f)
                        vh = vpool.tile([16, VCHUNK // 2], F32R, tag="vch", bufs=2)
                        nc.sync.dma_start(vh[:, :w_],
                                          vpack[:, base + hidx * half_f:base + hidx * half_f + w_])
                        for oo in range(0, w_, PT):
                            Fs = min(PT, w_ - oo)
                            Fs2 = Fs - (Fs & 3)
                            o0 = hidx * half_f + oo
                            ps = pmain.tile([6 * C, PT], F32, space="PSUM", tag="wps",
                                            bufs=1, name="wps")
                            nc.tensor.matmul(ps[:, :Fs], ulhsT[:],
                                             vh[:, oo:oo + Fs], start=True, stop=True)
                            if subi % 2 == 0:
                                nc.vector.tensor_copy(stg[:, o0:o0 + Fs2], ps[:, 0:Fs2])
                                if Fs2 < Fs:
                                    nc.scalar.activation(stg[:, o0 + Fs2:o0 + Fs],
                                                         ps[:, Fs2:Fs], AF.Copy)
                            else:
                                nc.scalar.activation(stg[:, o0:o0 + Fs], ps[:, 0:Fs], AF.Copy)
                            subi += 1
                    nc.sync.dma_start(stg_d[:, ci * VCHUNK:ci * VCHUNK + F], stg[:, :F])
                # whole-k rearrange DMAs (stg_d cols are globally (k,j)-contiguous)
                for ch in range(C):
                    srcs = []
                    for m in range(6):
                        row = 8 * m + ch
                        srcs.append(stg_d[row:row + 1, :HID * HID].rearrange(
                            "p (k g) -> (p k) g", k=HID))
                    for m in range(6):
                        nc.sync.dma_start(Whi[ch][m][:, :], srcs[m][0:HH, 0:HH])
                    for o, (ma, mb) in enumerate(O_MAP):
                        nc.sync.dma_start(Wpk[ch][o][0:HL, :], srcs[ma][HH:HID, 0:HH])
                        nc.sync.dma_start(Wpk[ch][o][64:64 + HL, :], srcs[mb][HH:HID, 0:HH])
                    for pr in range(2):
                        ma, mb, mn = (0, 1, 2) if pr == 0 else (3, 4, 5)
                        nc.sync.dma_start(WlpRe[ch][pr][:, 0:HL], srcs[ma][0:HH, HH:HID])
                        nc.sync.dma_start(WlpRe[ch][pr][:, 64:64 + HL], srcs[mb][0:HH, HH:HID])
                        nc.sync.dma_start(WlpIm[ch][pr][:, 0:HL], srcs[mn][0:HH, HH:HID])
                        nc.sync.dma_start(WlpIm[ch][pr][:, 64:64 + HL], srcs[ma][0:HH, HH:HID])
                        nc.sync.dma_start(WlpLo[ch][pr][0:HL, 0:HL], srcs[ma][HH:HID, HH:HID])
                        nc.sync.dma_start(WlpLo[ch][pr][0:HL, 64:64 + HL], srcs[mb][HH:HID, HH:HID])
                        nc.sync.dma_start(WlpLo[ch][pr][64:64 + HL, 0:HL], srcs[mn][HH:HID, HH:HID])
                        nc.sync.dma_start(WlpLo[ch][pr][64:64 + HL, 64:64 + HL], srcs[ma][HH:HID, HH:HID])

                # bias rows into Wpk row 117 (j 0:128) and WlpLo row 117 (j 128:181)
                for ch in range(C):
                    for o, (ma, mb) in enumerate(O_MAP):
                        nc.gpsimd.dma_start(Wpk[ch][o][117:118, :],
                                            b1g[ch:ch + 1, ma * HID:ma * HID + HH])
                    for pr in range(2):
                        ma, mb = (0, 1) if pr == 0 else (3, 4)
                        nc.gpsimd.dma_start(WlpLo[ch][pr][117:118, 0:HL],
                                            b1g[ch:ch + 1, ma * HID + HH:(ma + 1) * HID])
                        nc.gpsimd.dma_start(WlpLo[ch][pr][117:118, 64:64 + HL],
                                            b1g[ch:ch + 1, mb * HID + HH:(mb + 1) * HID])

                # Wf tiles
                wfg_d = dpool.tile([C, 2 * 182 * 6], F16)
                nc.sync.dma_start(wfg_d[:], wfg[:])
                for ch in range(C):
                    for a in range(2):
                        o = a * 182 * 6
                        nc.gpsimd.dma_start(
                            WfHi[ch][a][:],
                            wfg_d[ch:ch + 1, o:o + HH * 6].rearrange("p (k g) -> (p k) g", k=HH))
                    nc.gpsimd.dma_start(
                        WfPk[ch][0:HL, :],
                        wfg_d[ch:ch + 1, HH * 6:HID * 6].rearrange("p (k g) -> (p k) g", k=HL))
                    nc.gpsimd.dma_start(
                        WfPk[ch][64:64 + HL, :],
                        wfg_d[ch:ch + 1, 182 * 6 + HH * 6:182 * 6 + HID * 6].rearrange(
                            "p (k g) -> (p k) g", k=HL))
                    nc.gpsimd.dma_start(
                        WfPk[ch][117:118, :],
                        wfg_d[ch:ch + 1, HID * 6:182 * 6])

            # ---------- main pipeline ----------
            ngroups = C // GROUP_SIZE

            def plane(pool, tag, rows, dtp=F16, bufs=None, cols=NPTS):
                return pool.tile([rows, cols], dtp, tag=tag, name=tag, bufs=bufs)

            hplanes = {}

            def emit_l0_exp(chans, grp):
                for ch in chans:
                    j, s_ = ch // 3, ch % 3
                    b = 32 * s_
                    qacc_h = plane(pl2, "qacc_h", HH, bufs=1)
                    qacc_l = plane(pl2, "qacc_l", HL, bufs=1)
                    qtmp_l = plane(pl2, "qtmp_l", HL, bufs=1)
                    sqp = plane(pl2, "sqp", HH, bufs=1)
                    arg_h = plane(pl2, "arg_h", HH, cols=2 * NPTS)
                    arg_l = plane(pl2, "arg_l", HL, cols=2 * NPTS)
                    e_h = plane(pl2, "e_h", HH)
                    e_l = plane(pl2, "e_l", HL)
                    lh = w0sb[j][b:b + 3, :]
                    for np2 in range(NT // 2):
                        pLo = pmain.tile([HH, 2 * PT], F32, space="PSUM", tag="pb", bufs=1, name="pb")
                        qsl = slice(np2 * 2 * PT, (np2 + 1) * 2 * PT)
                        for half in range(2):
                            nt = np2 * 2 + half
                            sl = slice(nt * PT, (nt + 1) * PT)
                            sl2 = slice(NPTS + nt * PT, NPTS + (nt + 1) * PT)
                            hsl = slice(half * PT, (half + 1) * PT)
                            pH = pmain.tile([HH, 2 * PT], F32, space="PSUM", tag="pa", bufs=1, name="pa")
                            rhs = xsb[j][b:b + 3, sl]
                            nc.tensor.matmul(pH[:, 0:PT], lh[:, 0:HH], rhs, start=True, stop=True)
                            nc.tensor.matmul(pH[:, PT:2 * PT], lh[:, HH:2 * HH], rhs, start=True, stop=True)
                            nc.tensor.matmul(pLo[0:117, hsl], lh[:, 256:373], rhs, start=True, stop=True)
                            so = half * 2 * PT
                            act(nc.scalar.activation(sqp[:, so:so + 2 * PT], pH[:], AF.Square,
                                                     scale=1.0 / OM))
                            nc.vector.add_range_wrap(arg_h[:, sl], pH[:, 0:PT], 0.0, PI, 2 * PI)
                            nc.vector.add_range_wrap(arg_h[:, sl2], pH[:, 0:PT], PI / 2, PI, 2 * PI)
                            nc.vector.add_range_wrap(arg_l[:, sl], pLo[0:HL, hsl], 0.0, PI, 2 * PI)
                            nc.vector.add_range_wrap(arg_l[:, sl2], pLo[0:HL, hsl], PI / 2, PI, 2 * PI)
                            nc.vector.tensor_add(qacc_h[:, sl], sqp[:, so:so + PT],
                                                 sqp[:, so + PT:so + 2 * PT])
                        act(nc.scalar.activation(qacc_l[:, qsl], pLo[0:HL, :], AF.Square,
                                                 scale=1.0 / OM))
                        act(nc.scalar.activation(qtmp_l[:, qsl], pLo[64:64 + HL, :], AF.Square,
                                                 scale=1.0 / OM))
                        nc.vector.tensor_add(qacc_l[:, qsl], qacc_l[:, qsl], qtmp_l[:, qsl])
                    act(nc.scalar.activation(e_h[:], qacc_h[:], AF.Exp, scale=-100.0))
                    act(nc.scalar.activation(e_l[:], qacc_l[:], AF.Exp, scale=-100.0))
                    grp[ch] = (arg_h, arg_l, e_h, e_l)

            def emit_l0_trig(chans, grp):
                for ch in chans:
                    arg_h, arg_l, e_h, e_l = grp[ch]
                    sc_h = plane(pl2, "sc_h", HH, bufs=1, cols=2 * NPTS)
                    sc_l = plane(pl2, "sc_l", HL, bufs=1, cols=2 * NPTS)
                    h0re_h = plane(pl1, "h0re_h", HH)
                    h0im_h = plane(pl1, "h0im_h", HH)
                    h0lo = plane(pl1, "h0lo", KP)
                    nc.vector.memset(h0lo[:], 1.0)
                    act(nc.scalar.activation(sc_h[:], arg_h[:], AF.Sin))
                    act(nc.scalar.activation(sc_l[:], arg_l[:], AF.Sin))
                    nc.vector.tensor_mul(h0re_h[:], e_h[:], sc_h[:, NPTS:])
                    nc.vector.tensor_mul(h0im_h[:], e_h[:], sc_h[:, 0:NPTS])
                    nc.vector.tensor_mul(h0lo[0:HL, :], e_l[:], sc_l[:, NPTS:])
                    nc.vector.tensor_mul(h0lo[64:64 + HL, :], e_l[:], sc_l[:, 0:NPTS])
                    hplanes[ch] = (h0re_h, h0im_h, h0lo)

            def emit_l1_exp(chans, grp1):
                for ch in chans:
                    h0re_h, h0im_h, h0lo = hplanes[ch]
                    qacc_h = plane(pl2, "qacc_h", HH, bufs=1)
                    qacc_l = plane(pl2, "qacc_l", HL, bufs=1)
                    qtmp_l = plane(pl2, "qtmp_l", HL, bufs=1)
                    sqp = plane(pl2, "sqp", HH, bufs=1)
                    sql_a = plane(pl2, "sql_a", HL, bufs=1, cols=NPTS // 2)
                    sql_b = plane(pl2, "sql_b", HL, bufs=1, cols=NPTS // 2)
                    arg_h = plane(pl2, "arg_h", HH, cols=2 * NPTS)
                    arg_l = plane(pl2, "arg_l", HL, cols=2 * NPTS)
                    e_h = plane(pl2, "e_h", HH)
                    e_l = plane(pl2, "e_l", HL)

                    def mm3(psum, o, rhs_sl):
                        m_a, m_b = O_MAP[o]
                        nc.tensor.matmul(psum, Whi[ch][m_a][:], h0re_h[:, rhs_sl],
                                         start=True, stop=False)
                        nc.tensor.matmul(psum, Whi[ch][m_b][:], h0im_h[:, rhs_sl],
                                         start=False, stop=False)
                        nc.tensor.matmul(psum, Wpk[ch][o][:], h0lo[:, rhs_sl],
                                         start=False, stop=True)

                    def mm3lo(psum, pr, rhs_sl):
                        nc.tensor.matmul(psum, WlpRe[ch][pr][:], h0re_h[:, rhs_sl],
                                         start=True, stop=False)
                        nc.tensor.matmul(psum, WlpIm[ch][pr][:], h0im_h[:, rhs_sl],
                                         start=False, stop=False)
                        nc.tensor.matmul(psum, WlpLo[ch][pr][:], h0lo[:, rhs_sl],
                                         start=False, stop=True)

                    for nt in range(NT):
                        sl = slice(nt * PT, (nt + 1) * PT)
                        sl2 = slice(NPTS + nt * PT, NPTS + (nt + 1) * PT)
                        pH = pmain.tile([HH, 2 * PT], F32, space="PSUM", tag="pa", bufs=1, name="pa")
                        pH2 = pmain.tile([HH, 2 * PT], F32, space="PSUM", tag="pc", bufs=1, name="pc")
                        pLo = pmain.tile([HH, 2 * PT], F32, space="PSUM", tag="pb", bufs=1, name="pb")
                        mm3(pH[:, 0:PT], 0, sl)
                        mm3(pH[:, PT:2 * PT], 1, sl)
                        mm3lo(pLo[0:HH, 0:PT], 0, sl)
                        mm3lo(pLo[0:HH, PT:2 * PT], 1, sl)
                        mm3(pH2[:, 0:PT], 2, sl)
                        mm3(pH2[:, PT:2 * PT], 3, sl)
                        act(nc.scalar.activation(sqp[:, 0:2 * PT], pH[:], AF.Square,
                                                 scale=1.0 / OM))
                        nc.vector.add_range_wrap(arg_h[:, sl], pH[:, 0:PT], 0.0, PI, 2 * PI)
                        nc.vector.add_range_wrap(arg_h[:, sl2], pH[:, 0:PT], PI / 2, PI, 2 * PI)
                        nc.vector.add_range_wrap(arg_l[:, sl], pLo[0:HL, 0:PT], 0.0, PI, 2 * PI)
                        nc.vector.add_range_wrap(arg_l[:, sl2], pLo[0:HL, 0:PT], PI / 2, PI, 2 * PI)
                        act(nc.scalar.activation(sql_a[:, :], pLo[0:HL, :], AF.Square,
                                                 scale=1.0 / OM))
                        act(nc.scalar.activation(sql_b[:, :], pLo[64:64 + HL, :], AF.Square,
                                                 scale=1.0 / OM))
                        act(nc.scalar.activation(sqp[:, 2 * PT:4 * PT], pH2[:], AF.Square,
                                                 scale=1.0 / OM))
                        nc.vector.tensor_add(qacc_h[:, sl], sqp[:, 0:PT], sqp[:, PT:2 * PT])
                        nc.vector.tensor_add(qacc_h[:, sl], qacc_h[:, sl], sqp[:, 2 * PT:3 * PT])
                        nc.vector.tensor_add(qacc_h[:, sl], qacc_h[:, sl], sqp[:, 3 * PT:4 * PT])
                        nc.vector.tensor_add(qacc_l[:, sl], sql_a[:, 0:PT], sql_a[:, PT:2 * PT])
                        nc.vector.tensor_add(qtmp_l[:, sl], sql_b[:, 0:PT], sql_b[:, PT:2 * PT])
                        nc.vector.tensor_add(qacc_l[:, sl], qacc_l[:, sl], qtmp_l[:, sl])
                    act(nc.scalar.activation(e_h[:], qacc_h[:], AF.Exp,
                                         scale=-100.0, bias=c_225[:HH, :1]))
                    act(nc.scalar.activation(e_l[:], qacc_l[:], AF.Exp,
                                         scale=-100.0, bias=c_225[:HL, :1]))
                    grp1[ch] = (arg_h, arg_l, e_h, e_l)

            def emit_l1_trig(chans, grp1):
                for ch in chans:
                    arg_h, arg_l, e_h, e_l = grp1[ch]
                    sc_h = plane(pl2, "sc_h", HH, bufs=1, cols=2 * NPTS)
                    sc_l = plane(pl2, "sc_l", HL, bufs=1, cols=2 * NPTS)
                    h1re_h = plane(pl1, "h0re_h", HH)
                    h1im_h = plane(pl1, "h0im_h", HH)
                    h1lo = plane(pl1, "h0lo", KP)
                    nc.vector.memset(h1lo[:], 1.0)
                    act(nc.scalar.activation(sc_h[:], arg_h[:], AF.Sin))
                    act(nc.scalar.activation(sc_l[:], arg_l[:], AF.Sin))
                    nc.vector.tensor_mul(h1re_h[:], e_h[:], sc_h[:, NPTS:])
                    nc.vector.tensor_mul(h1im_h[:], e_h[:], sc_h[:, 0:NPTS])
                    nc.vector.tensor_mul(h1lo[0:HL, :], e_l[:], sc_l[:, NPTS:])
                    nc.vector.tensor_mul(h1lo[64:64 + HL, :], e_l[:], sc_l[:, 0:NPTS])
                    for nt in range(NT):
                        sl = slice(nt * PT, (nt + 1) * PT)
                        pf = pmain.tile([6, PT], F32, space="PSUM", tag="fin", bufs=1, name="pf")
                        nc.tensor.matmul(pf[:], WfHi[ch][0][:], h1re_h[:, sl], start=True, stop=False)
                        nc.tensor.matmul(pf[:], WfHi[ch][1][:], h1im_h[:, sl], start=False, stop=False)
                        nc.tensor.matmul(pf[:], WfPk[ch][:], h1lo[:, sl], start=False, stop=True)
                        fs = fpool.tile([6, PT], F32, tag="fstage")
                        nc.vector.tensor_copy(fs[:], pf[:])
                        nc.sync.dma_start(out48[6 * ch:6 * ch + 6, sl], fs[:])

            # emission order: g0 L0 -> W-build -> g0 trig/L1/... -> g1 ...
            groups = [list(range(g * GROUP_SIZE, (g + 1) * GROUP_SIZE))
                      for g in range(ngroups)]
            grps = [dict() for _ in range(ngroups)]
            grps1 = [dict() for _ in range(ngroups)]

            emit_l0_exp(groups[0], grps[0])
            emit_wbuild()
            new_phase()
            emit_l0_trig(groups[0], grps[0])
            for g in range(ngroups):
                if g > 0:
                    new_phase()
                    emit_l0_exp(groups[g], grps[g])
                    new_phase()
                    emit_l0_trig(groups[g], grps[g])
                new_phase()
                emit_l1_exp(groups[g], grps1[g])
                new_phase()
                emit_l1_trig(groups[g], grps1[g])

            # ---------- ACT phase barriers (force table-set grouping) ----------
            bars = []
            for p in range(0):
                if p % 2 == 0:
                    b = nc.scalar.copy(bar_b[:], bar_a[:])
                else:
                    b = nc.scalar.copy(bar_a[:], bar_b[:])
                bars.append(b)
            for p, b in enumerate(bars):
                for op in acts_by_phase[p]:
                    add_dep_helper(b.ins, op.ins, sync=False, reason=f"phase{p}end")
                for op in acts_by_phase[p + 1]:
                    add_dep_helper(op.ins, b.ins, sync=False, reason=f"phase{p+1}start")

    nc.compile()
    return nc


def _prep(inputs):
    x = np.ascontiguousarray(inputs["x"], dtype=np.float32)
    indices = np.ascontiguousarray(inputs["indices"], dtype=np.int32)
    w0_lin = np.asarray(inputs["w0_lin"], dtype=np.float32)
    b0_lin = np.asarray(inputs["b0_lin"], dtype=np.float32)
    w0_orth = np.asarray(inputs["w0_orth"], dtype=np.float32)
    b0_orth = np.asarray(inputs["b0_orth"], dtype=np.float32)
    U1_lin = np.asarray(inputs["U1_lin"], dtype=np.complex64)
    V1_lin = np.asarray(inputs["V1_lin"], dtype=np.complex64)
    b1_lin = np.asarray(inputs["b1_lin"], dtype=np.complex64)
    U1_orth = np.asarray(inputs["U1_orth"], dtype=np.complex64)
    V1_orth = np.asarray(inputs["V1_orth"], dtype=np.complex64)
    b1_orth = np.asarray(inputs["b1_orth"], dtype=np.complex64)
    Wf = np.asarray(inputs["Wf"], dtype=np.complex64)
    bf = np.asarray(inputs["bf"], dtype=np.complex64)

    w0pack = np.zeros((NCH, 3, 384), np.float32)
    w0pack[:, 0:2, 0:HH] = w0_lin[:, :, 0:HH]
    w0pack[:, 2, 0:HH] = b0_lin[:, 0, 0:HH]
    w0pack[:, 0:2, 128:128 + HH] = w0_orth[:, :, 0:HH]
    w0pack[:, 2, 128:128 + HH] = b0_orth[:, 0, 0:HH]
    w0pack[:, 0:2, 256:256 + HL] = w0_lin[:, :, HH:HID]
    w0pack[:, 2, 256:256 + HL] = b0_lin[:, 0, HH:HID]
    w0pack[:, 0:2, 320:320 + HL] = w0_orth[:, :, HH:HID]
    w0pack[:, 2, 320:320 + HL] = b0_orth[:, 0, HH:HID]
    w0pack = (30.0 * w0pack).reshape(NCH, 3 * 384).astype(np.float16)

    up = np.concatenate([U1_lin.real, U1_lin.imag, U1_orth.real, U1_orth.imag],
                        axis=1).astype(np.float32)  # (128, 16)
    upack = np.concatenate([up, -up], axis=1)  # (128, 32)
    vpack = np.concatenate([V1_lin.real, V1_lin.imag, V1_orth.real, V1_orth.imag],
                           axis=0).astype(np.float32)  # (16, 32761)
    vpack = np.concatenate([vpack, np.zeros((16, 1), np.float32)], axis=1)
    z = np.zeros_like(b1_lin[:, 0, :].real)
    b1pack = 30.0 * np.concatenate(
        [b1_lin[:, 0, :].real, b1_lin[:, 0, :].imag + 0.15, z,
         b1_orth[:, 0, :].real, b1_orth[:, 0, :].imag, z],
        axis=1).astype(np.float16)  # (128, 1086)

    wfpack = np.zeros((NCH, 2, 182, 6), np.float32)
    wfpack[:, 0, 0:HID, 0:3] = Wf.real
    wfpack[:, 0, 0:HID, 3:6] = Wf.imag
    wfpack[:, 0, HID, 0:3] = bf[:, 0, :].real
    wfpack[:, 0, HID, 3:6] = bf[:, 0, :].imag
    wfpack[:, 1, 0:HID, 0:3] = -Wf.imag
    wfpack[:, 1, 0:HID, 3:6] = Wf.real
    wfpack = wfpack.reshape(NCH, 2 * 182 * 6).astype(np.float16)

    in_maps = []
    for core in range(NCORES):
        c0 = core * C
        xs = x[c0:c0 + C]
        xpk = [np.zeros((67, NPTS), np.float16) for _ in range(3)]
        for ch in range(C):
            j, s = ch // 3, ch % 3
            xpk[j][32 * s:32 * s + 2, :] = xs[ch].T
            xpk[j][32 * s + 2, :] = 1.0
        m = {f"xpk{j}": xpk[j] for j in range(3)}
        m["idx"] = indices[c0:c0 + C].reshape(C, 1)
        m["w0pack"] = w0pack
        m["upack"] = upack
        m["vpack"] = vpack
        m["b1pack"] = b1pack
        m["wfpack"] = wfpack
        in_maps.append(m)
    return in_maps


def kernel(**inputs):
    from concourse import bass_utils
    if "nc" not in _CACHE:
        _CACHE["nc"] = _build()
    nc = _CACHE["nc"]
    in_maps = _prep(inputs)
    res = bass_utils.run_bass_kernel_spmd(nc, in_maps, core_ids=list(range(NCORES)))
    out = np.zeros((NSEL, NPTS, OUT), np.complex64)
    for core in range(NCORES):
        o = res.results[core]["out48"]
        for ch in range(C):
            re = o[6 * ch:6 * ch + 3, :]
            im = o[6 * ch + 3:6 * ch + 6, :]
            out[core * C + ch] = (re + 1j * im).T.astype(np.complex64)
    return out
